# revision 1
# baseline (speedup 1.0000x reference)
"""Trainium2 Bass kernel for GCFAgg-style block:
    q1 = x@W1.T+b1; q2 = x@W2.T+b2; r = x@WR.T+br
    out = (q1 @ q2.T) @ r        (per batch, no softmax)

Key algebraic restructuring: with x_aug = [x | 1] and W*_aug = [W* | b*],
    out = x_aug @ (Khat @ (x_aug.T @ x_aug) @ Rhat)
where Khat = W1_aug.T @ W2_aug and Rhat = WR_aug.T are tiny host-precomputed
matrices. The device only computes G = x.T @ x (per batch, symmetric — only
upper blocks are computed, lower blocks come from PE transposes) plus a small
[640]^2-sized chain and the final projection out = x @ P + v. That's
~3.5 GFLOP/core instead of ~41 GFLOP/core for the naive N x N similarity
materialization. The augmented row/col of G (column sums of x) and the
constant v-broadcast are folded in from host-precomputed side inputs.

Numerics: fp32r matmuls (fp32 storage, single-pass reduced-precision PE
multiply) giving ~2e-4 relative error vs the fp32 reference — ~10x tighter
than bf16 at only ~10-15% more device time.

Sharding: batch dim B=8, one batch per NeuronCore (data parallel, 8 cores).

Self-contained: hardcodes shapes from the problem spec
(x: [8, 4096, 512] f32; W*: [512, 512]; b*: [512]).
"""
import os
import sys

sys.path.insert(0, "/opt/trn_rl_repo")

import numpy as np
import ml_dtypes

import concourse.bass as bass
import concourse.mybir as mybir
import concourse.tile as tile
from concourse import bacc
from concourse.bass_utils import run_bass_kernel_spmd
from concourse.masks import make_identity
from concourse.tile_rust import add_dep_helper

B = 8          # batch -> one per core
N = 4096       # tokens per batch
D = 512        # model dim
GP = 640       # augmented dim 513 padded to 5*128
NCHUNK = GP // 128   # 5
NT = N // 128        # 32 row tiles
N_CORES = 8

F32 = mybir.dt.float32
F32R = mybir.dt.float32r
BF16 = mybir.dt.bfloat16

# mode: "f32r" (fp32 storage, single-pass reduced-precision matmul),
#       "bf16" (bf16 storage+matmul), "f32" (full-precision 4-pass matmul)
MODE = os.environ.get("GCF_MODE", "f32r")

_built = {}


def _build(mode):
    if mode in _built:
        return _built[mode]

    # Storage dtype IS the matmul dtype: the BIR verifier requires fp32r
    # matmul inputs to be produced (DMA'd/copied) as fp32r.
    if mode == "bf16":
        big_mm = BF16
    elif mode == "f32":
        big_mm = F32
    else:
        big_mm = F32R
    big_store = big_mm
    chain_mm = F32 if mode == "f32" else F32R

    def mm_ap(ap, dt):
        return ap if ap.dtype == dt else ap.bitcast(dt)

    nc = bacc.Bacc("TRN2", target_bir_lowering=False, debug=False,
                   num_devices=N_CORES)

    xa_d = nc.dram_tensor("xa", (N, D), big_store, kind="ExternalInput")
    xat_d = nc.dram_tensor("xat", (NT, 128, 4, 128), big_store,
                           kind="ExternalInput")
    khatT_d = nc.dram_tensor("khatT", (GP, GP), chain_mm, kind="ExternalInput")
    rhat_d = nc.dram_tensor("rhat", (GP, D), chain_mm, kind="ExternalInput")
    # host-precomputed augmented pieces of G_aug (they only involve column
    # sums of x, cheap on host): rows 512:640, and the [:, 512:640] blocks
    gext_d = nc.dram_tensor("gext", (128, GP), chain_mm, kind="ExternalInput")
    augblk_d = nc.dram_tensor("augblk", (4, 128, GP - D), chain_mm,
                              kind="ExternalInput")
    m1row_d = nc.dram_tensor("m1row", (1, D), chain_mm, kind="ExternalInput")
    out_d = nc.dram_tensor("out", (N, D), F32, kind="ExternalOutput")

    with tile.TileContext(nc) as tc:
        with (
            tc.tile_pool(name="xa", bufs=16) as xa_pool,
            tc.tile_pool(name="xat", bufs=32) as xat_pool,
            tc.tile_pool(name="const", bufs=1) as const_pool,
            tc.tile_pool(name="gsb", bufs=1) as g_pool,
            tc.tile_pool(name="chain", bufs=1) as chain_pool,
            tc.tile_pool(name="outsb", bufs=6) as out_pool,
        ):
            # ---- constants (via the otherwise-idle GpSimd DMA queue so they
            # don't delay the sync-queue xa/xat streams) ----
            khat_sb = [const_pool.tile([128, GP], chain_mm, tag=f"khat{c}",
                                       name=f"khat{c}") for c in range(NCHUNK)]
            rhat_sb = [const_pool.tile([128, D], chain_mm, tag=f"rhat{c}",
                                       name=f"rhat{c}") for c in range(NCHUNK)]
            ident = const_pool.tile([128, 128], F32, tag="ident")
            make_identity(nc, ident[:])
            # dtype for the K=1 v-broadcast matmul: bitcasting f32r->f32 is
            # size-preserving, but bf16 tiles must stay bf16 (memset can emit
            # bf16/f32, just not f32r)
            v_mm_dt = big_mm if big_mm == BF16 else F32
            ones_row = const_pool.tile([1, 128], v_mm_dt, tag="ones_row")
            nc.vector.memset(ones_row[:], 1.0)

            # ---- phase 1: G = x^T @ x over 32 row tiles; G is symmetric so
            # only the upper block-triangle is computed on PE ----
            g_sb = [g_pool.tile([128, GP], chain_mm, tag=f"g{c}", name=f"g{c}")
                    for c in range(NCHUNK)]
            with tc.tile_pool(name="psG", bufs=1, space="PSUM") as psG_pool:
                ps_ga = [psG_pool.tile([128, D - c * 128], F32, tag=f"ga{c}",
                                       name=f"ga{c}") for c in range(4)]
                gate_mms = []
                for t in range(NT):
                    xa_t = xa_pool.tile([128, D], big_store, tag="xa")
                    nc.sync.dma_start(xa_t[:], xa_d.ap()[t * 128:(t + 1) * 128, :])
                    for c in range(4):
                        mm = nc.tensor.matmul(
                            ps_ga[c][:],
                            mm_ap(xa_t[:, c * 128:(c + 1) * 128], big_mm),
                            mm_ap(xa_t[:, c * 128:D], big_mm),
                            start=(t == 0), stop=(t == NT - 1),
                        )
                        if c == 3:
                            gate_mms.append(mm)
                gate_mm16 = gate_mms[16]
                # constants + host-side G_aug pieces: gated behind mid-G so
                # their DMAs don't compete with the xa stream during warmup
                # (they're first needed at chain time)
                const_dmas = []
                const_dmas.append(nc.gpsimd.dma_start(g_sb[4][:], gext_d.ap()[:]))
                for c in range(4):
                    const_dmas.append(
                        nc.gpsimd.dma_start(g_sb[c][:, D:GP], augblk_d.ap()[c]))
                m1row_sb = const_pool.tile([1, D], chain_mm, tag="m1row")
                const_dmas.append(nc.gpsimd.dma_start(m1row_sb[:], m1row_d.ap()[:]))
                for c in range(NCHUNK):
                    const_dmas.append(nc.gpsimd.dma_start(
                        khat_sb[c][:], khatT_d.ap()[c * 128:(c + 1) * 128, :]))
                    const_dmas.append(nc.gpsimd.dma_start(
                        rhat_sb[c][:], rhat_d.ap()[c * 128:(c + 1) * 128, :]))
                for cd in const_dmas:
                    add_dep_helper(cd.ins, gate_mm16.ins,
                                   reason="const loads gated behind G t=16")
                # upper blocks into SBUF
                for c in range(4):
                    nc.vector.tensor_copy(g_sb[c][:, c * 128:D], ps_ga[c][:])
                # lower blocks = transpose of upper (G symmetric)
                for c2 in range(1, 4):
                    for c1 in range(c2):
                        ps_tr = psG_pool.tile([128, 128], F32, tag="tr", bufs=2)
                        nc.tensor.transpose(
                            ps_tr[:],
                            mm_ap(g_sb[c1][:, c2 * 128:(c2 + 1) * 128], F32),
                            ident[:],
                        )
                        nc.vector.tensor_copy(
                            g_sb[c2][:, c1 * 128:(c1 + 1) * 128], ps_tr[:])

            # ---- phase 2: P = Khat @ G @ Rhat  (small chain) ----
            with tc.tile_pool(name="psC", bufs=2, space="PSUM") as psC_pool:
                # M1 rows 512:640 come from host (m1row = sx_aug @ Rhat);
                # device computes chunks 0..3 only
                m1_sb = [chain_pool.tile([128, D], chain_mm, tag=f"m1{c}",
                                         name=f"m1{c}") for c in range(4)]
                for g1 in range(4):
                    ps = psC_pool.tile([128, D], F32, tag="chain", bufs=3)
                    for g2 in range(NCHUNK):
                        nc.tensor.matmul(
                            ps[:],
                            mm_ap(g_sb[g2][:, g1 * 128:(g1 + 1) * 128], chain_mm),
                            mm_ap(rhat_sb[g2][:], chain_mm),
                            start=(g2 == 0), stop=(g2 == NCHUNK - 1),
                        )
                    nc.vector.tensor_copy(m1_sb[g1][:], ps[:])

                p_sb = [chain_pool.tile([128, D], big_store, tag=f"p{c}",
                                        name=f"p{c}") for c in range(NCHUNK)]
                for g1 in range(NCHUNK):
                    ps = psC_pool.tile([128, D], F32, tag="chain", bufs=3)
                    for g2 in range(4):
                        nc.tensor.matmul(
                            ps[:],
                            mm_ap(khat_sb[g2][:, g1 * 128:(g1 + 1) * 128], chain_mm),
                            mm_ap(m1_sb[g2][:], chain_mm),
                            start=(g2 == 0), stop=False,
                        )
                    # g2=4 contribution: only row 512 of K^T/M1 is nonzero
                    nc.tensor.matmul(
                        ps[:],
                        mm_ap(khat_sb[4][0:1, g1 * 128:(g1 + 1) * 128], chain_mm),
                        mm_ap(m1row_sb[0:1, :], chain_mm),
                        start=False, stop=True,
                    )
                    nc.vector.tensor_copy(p_sb[g1][:], ps[:])

            # ---- phase 3: out = x @ P[0:512] + v,  v = P_aug[512, :] ----
            with tc.tile_pool(name="psO", bufs=1, space="PSUM") as psO_pool:
                # v broadcast to 128 partitions via a K=1 fp32 matmul
                ps_v = psO_pool.tile([128, D], F32, tag="v", bufs=1)
                nc.tensor.matmul(
                    ps_v[:], ones_row[0:1, :], mm_ap(p_sb[4][0:1, :], v_mm_dt),
                    start=True, stop=True,
                )
                v_sb = const_pool.tile([128, D], F32, tag="vsb")
                nc.vector.tensor_copy(v_sb[:], ps_v[:])

                for t in range(NT):
                    xat_t = xat_pool.tile([128, 4, 128], big_store, tag="xat")
                    xdma = nc.scalar.dma_start(xat_t[:], xat_d.ap()[t])
                    # full xat residency, bandwidth-shaped: the xa stream alone
                    # needs ~190GB/s of the ~340GB/s during G, so release xat
                    # at only 1 tile per 2 G tiles there; the remainder streams
                    # during the chain window, which otherwise runs at ~60% BW
                    add_dep_helper(xdma.ins, gate_mms[min(NT - 1, 2 * t + 6)].ins,
                                   reason="xat prefetch BW-shaped behind G")
                    ps = psO_pool.tile([128, D], F32, tag="out", bufs=6)
                    for c in range(4):
                        nc.tensor.matmul(
                            ps[:],
                            mm_ap(xat_t[:, c, :], big_mm),
                            mm_ap(p_sb[c][:], big_mm),
                            start=(c == 0), stop=(c == 3),
                        )
                    ot = out_pool.tile([128, D], F32, tag="ot")
                    nc.vector.tensor_add(ot[:], ps[:], v_sb[:])
                    # alternate store triggers across two queues: a single
                    # queue serializes 32 x ~640ns DMA_DIRECT2D triggers
                    eng = nc.gpsimd if t % 2 == 0 else nc.sync
                    eng.dma_start(out_d.ap()[t * 128:(t + 1) * 128, :], ot[:])

    nc.compile()
    _built[mode] = nc
    return nc


def _prep_host(x, Wq1_w, Wq1_b, Wq2_w, Wq2_b, WR_w, WR_b, mode):
    f = np.float32
    W1a = np.concatenate([Wq1_w, Wq1_b[:, None]], axis=1)   # [512, 513]
    W2a = np.concatenate([Wq2_w, Wq2_b[:, None]], axis=1)
    WRa = np.concatenate([WR_w, WR_b[:, None]], axis=1)

    khatT = np.zeros((GP, GP), f)   # Khat^T = W2a^T @ W1a, padded
    khatT[:D + 1, :D + 1] = (
        W2a.T.astype(np.float64) @ W1a.astype(np.float64)
    ).astype(f)
    rhat = np.zeros((GP, D), f)     # Rhat = WRa^T, padded
    rhat[:D + 1, :] = WRa.T

    # augmented pieces of G_aug = xa^T @ xa that only need column sums of x
    sx = x.sum(axis=1, dtype=np.float64).astype(f)       # [B, 512]
    gext = np.zeros((B, 128, GP), f)                     # G_aug rows 512:640
    gext[:, 0, :D] = sx
    gext[:, 0, D] = float(N)
    augblk = np.zeros((B, 4, 128, GP - D), f)            # G_aug[:512, 512:640]
    augblk[:, :, :, 0] = sx.reshape(B, 4, 128)
    # M1 row 512 = sx_aug @ Rhat (fully host-computable)
    sxa = np.concatenate([sx, np.full((B, 1), float(N), f)], axis=1)  # [B, 513]
    m1row = (sxa.astype(np.float64) @ WRa.T.astype(np.float64)).astype(f)[:, None, :]

    # xat[b, t, p, c, j] = x[b, t*128+j, c*128+p] — per-(t) contiguous
    # [128, 4, 128] lhsT blocks of x^T
    xat = np.ascontiguousarray(
        x.transpose(0, 2, 1)                     # [B, 512, 4096]
         .reshape(B, 4, 128, NT, 128)            # [B, c, p, t, j]
         .transpose(0, 3, 2, 1, 4)               # [B, t, p, c, j]
    )
    xa = x

    if mode == "bf16":
        bf = ml_dtypes.bfloat16
        xa = xa.astype(bf)
        xat = xat.astype(bf)
    else:
        xa = np.ascontiguousarray(xa)
    return xa, xat, khatT, rhat, gext, augblk, m1row


def kernel(x, Wq1_w, Wq1_b, Wq2_w, Wq2_b, WR_w, WR_b):
    x = np.asarray(x, dtype=np.float32)
    args = [np.asarray(a, dtype=np.float32)
            for a in (Wq1_w, Wq1_b, Wq2_w, Wq2_b, WR_w, WR_b)]
    xa, xat, khatT, rhat, gext, augblk, m1row = _prep_host(x, *args, MODE)

    nc = _build(MODE)
    in_maps = [
        {"xa": xa[b], "xat": xat[b], "khatT": khatT, "rhat": rhat,
         "gext": gext[b], "augblk": augblk[b], "m1row": m1row[b]}
        for b in range(B)
    ]
    # the axon-tunneled device occasionally starts in a wedged state
    # (NRT_EXEC_UNIT_UNRECOVERABLE) and recovers on the next attempt
    last_err = None
    for attempt in range(3):
        try:
            res = run_bass_kernel_spmd(nc, in_maps, core_ids=list(range(N_CORES)))
            break
        except Exception as e:  # noqa: BLE001
            last_err = e
            import time as _time
            _time.sleep(2.0)
            try:
                import jax
                jax.clear_caches()
            except Exception:
                pass
    else:
        raise last_err
    return np.stack([res.results[b]["out"] for b in range(B)])



# revision 2
# speedup vs baseline: 1.2031x; 1.2031x over previous
"""Trainium2 Bass kernel for GCFAgg-style block:
    q1 = x@W1.T+b1; q2 = x@W2.T+b2; r = x@WR.T+br
    out = (q1 @ q2.T) @ r        (per batch, no softmax)

Algebraic restructuring: with x_aug = [x | 1] and W*_aug = [W* | b*],
    out = x_aug @ P_full,  P_full = Khat @ G_aug @ Rhat
where Khat = W1_aug.T @ W2_aug, Rhat = WR_aug.T. The device computes only
the core G = x.T @ x (symmetric: upper blocks on PE, lower via PE
transposes); every term of P_full involving the augmented row/col of
G_aug is a host-computable rank-1 correction, folded into one K=2 matmul
per P chunk (lhsT2/rhs2 below). Final projection: out = x @ P[0:512] + v
with v = P_full[512].

Per-core work: ~2 N D^2 MACs (the algorithmic minimum for this
factorization) vs ~41 GFLOP for the naive N x N similarity.

Numerics: bf16 storage + matmuls with f32 PSUM accumulation; output
stored bf16 and upcast on host. End-to-end ~5e-3 max-rel error vs the
fp32 reference (gate is 2e-2).

Sharding: batch dim B=8, one batch per NeuronCore (data parallel).

Self-contained: hardcodes shapes (x: [8, 4096, 512] f32).
"""
import os
import sys

sys.path.insert(0, "/opt/trn_rl_repo")

import numpy as np
import ml_dtypes

import concourse.bass as bass
import concourse.mybir as mybir
import concourse.tile as tile
from concourse import bacc
from concourse.bass_utils import run_bass_kernel_spmd
from concourse.masks import make_identity
from concourse.tile_rust import add_dep_helper

B = 8          # batch -> one per core
N = 4096       # tokens per batch
D = 512        # model dim
GP = 640       # augmented dim 513 padded to 5*128 (khat col pad)
NT = N // 128  # 32 row tiles
N_CORES = 8

F32 = mybir.dt.float32
BF16 = mybir.dt.bfloat16
BF = ml_dtypes.bfloat16

_built = {}


def _build(key="v2"):
    if key in _built:
        return _built[key]

    nc = bacc.Bacc("TRN2", target_bir_lowering=False, debug=False,
                   num_devices=N_CORES)

    xa_d = nc.dram_tensor("xa", (N, D), BF16, kind="ExternalInput")
    xat_d = nc.dram_tensor("xat", (NT, 128, 4, 128), BF16,
                           kind="ExternalInput")
    khatT_d = nc.dram_tensor("khatT", (4, 128, GP), BF16, kind="ExternalInput")
    rhat_d = nc.dram_tensor("rhat", (4, 128, D), BF16, kind="ExternalInput")
    lhsT2_d = nc.dram_tensor("lhsT2", (2, GP), BF16, kind="ExternalInput")
    rhs2_d = nc.dram_tensor("rhs2", (2, D), BF16, kind="ExternalInput")
    out_d = nc.dram_tensor("out", (N, D), BF16, kind="ExternalOutput")

    with tile.TileContext(nc) as tc:
        with (
            tc.tile_pool(name="xa", bufs=16) as xa_pool,
            tc.tile_pool(name="xat", bufs=32) as xat_pool,
            tc.tile_pool(name="const", bufs=1) as const_pool,
            tc.tile_pool(name="gsb", bufs=1) as g_pool,
            tc.tile_pool(name="chain", bufs=1) as chain_pool,
            tc.tile_pool(name="outsb", bufs=6) as out_pool,
        ):
            khat_sb = [const_pool.tile([128, GP], BF16, tag=f"khat{c}",
                                       name=f"khat{c}") for c in range(4)]
            rhat_sb = [const_pool.tile([128, D], BF16, tag=f"rhat{c}",
                                       name=f"rhat{c}") for c in range(4)]
            lhsT2_sb = const_pool.tile([2, GP], BF16, tag="lhsT2")
            rhs2_sb = const_pool.tile([2, D], BF16, tag="rhs2")
            ident = const_pool.tile([128, 128], BF16, tag="ident")
            make_identity(nc, ident[:])
            ones_row = const_pool.tile([1, 128], BF16, tag="ones_row")
            nc.vector.memset(ones_row[:], 1.0)

            # ---- phase 1: G = x^T @ x over 32 row tiles (upper blocks) ----
            # xa tiles stream on two DMA queues (sync/gpsimd) for bandwidth
            g_sb = [g_pool.tile([128, D], BF16, tag=f"g{c}", name=f"g{c}")
                    for c in range(4)]
            with tc.tile_pool(name="psG", bufs=1, space="PSUM") as psG_pool:
                ps_ga = [psG_pool.tile([128, D - c * 128], F32, tag=f"ga{c}",
                                       name=f"ga{c}") for c in range(4)]
                gate_mms = []
                for t in range(NT):
                    xa_t = xa_pool.tile([128, D], BF16, tag="xa")
                    eng = nc.sync if t % 2 == 0 else nc.gpsimd
                    eng.dma_start(xa_t[:], xa_d.ap()[t * 128:(t + 1) * 128, :])
                    for c in range(4):
                        mm = nc.tensor.matmul(
                            ps_ga[c][:],
                            xa_t[:, c * 128:(c + 1) * 128],
                            xa_t[:, c * 128:D],
                            start=(t == 0), stop=(t == NT - 1),
                        )
                        if c == 3:
                            gate_mms.append(mm)
                # constants: gated behind early G so their triggers queue
                # after the gpsimd xa triggers and don't delay warmup
                const_dmas = [
                    nc.gpsimd.dma_start(lhsT2_sb[:], lhsT2_d.ap()[:]),
                    nc.gpsimd.dma_start(rhs2_sb[:], rhs2_d.ap()[:]),
                ]
                for c in range(4):
                    const_dmas.append(
                        nc.gpsimd.dma_start(khat_sb[c][:], khatT_d.ap()[c]))
                    const_dmas.append(
                        nc.gpsimd.dma_start(rhat_sb[c][:], rhat_d.ap()[c]))
                for cd in const_dmas:
                    add_dep_helper(cd.ins, gate_mms[8].ins,
                                   reason="const loads gated behind G t=8")
                # upper blocks into SBUF (cast f32 PSUM -> bf16)
                for c in range(4):
                    nc.vector.tensor_copy(g_sb[c][:, c * 128:D], ps_ga[c][:])
                # lower blocks = transpose of upper (G symmetric)
                for c2 in range(1, 4):
                    for c1 in range(c2):
                        ps_tr = psG_pool.tile([128, 128], BF16, tag="tr",
                                              bufs=2)
                        nc.tensor.transpose(
                            ps_tr[:],
                            g_sb[c1][:, c2 * 128:(c2 + 1) * 128],
                            ident[:],
                        )
                        nc.vector.tensor_copy(
                            g_sb[c2][:, c1 * 128:(c1 + 1) * 128], ps_tr[:])

            # ---- phase 2: P_full = Khat @ G_aug @ Rhat ----
            # core chain: M1[j] = sum_k G[kblk, jblk]^T Rhat[kblk]
            #             P[i]  = sum_j Khat[iblk, jblk] M1[j]  (+ K=2 aug)
            with tc.tile_pool(name="psC", bufs=1, space="PSUM") as psC_pool:
                m1_sb = [chain_pool.tile([128, D], BF16, tag=f"m1{c}",
                                         name=f"m1{c}") for c in range(4)]
                ps_p = [psC_pool.tile([128, D], F32, tag=f"pp{c}",
                                      name=f"pp{c}") for c in range(4)]
                ps_v = psC_pool.tile([128, D], F32, tag="pv", name="pv")
                for j in range(4):
                    ps = psC_pool.tile([128, D], F32, tag="m1ps", bufs=2)
                    for k in range(4):
                        nc.tensor.matmul(
                            ps[:], g_sb[k][:, j * 128:(j + 1) * 128],
                            rhat_sb[k][:],
                            start=(k == 0), stop=(k == 3),
                        )
                    nc.vector.tensor_copy(m1_sb[j][:], ps[:])
                    # interleave P's j-step right after M1[j] is available
                    for i in range(4):
                        nc.tensor.matmul(
                            ps_p[i][:],
                            khat_sb[j][:, i * 128:(i + 1) * 128],
                            m1_sb[j][:],
                            start=(j == 0), stop=False,
                        )
                    nc.tensor.matmul(
                        ps_v[0:1, :], khat_sb[j][:, 512:513], m1_sb[j][:],
                        start=(j == 0), stop=False,
                    )
                # K=2 augmented-rank-1 fold (host-precomputed lhsT2/rhs2)
                p_sb = [chain_pool.tile([128, D], BF16, tag=f"p{c}",
                                        name=f"p{c}") for c in range(4)]
                for i in range(4):
                    nc.tensor.matmul(
                        ps_p[i][:], lhsT2_sb[:, i * 128:(i + 1) * 128],
                        rhs2_sb[:], start=False, stop=True,
                    )
                    nc.vector.tensor_copy(p_sb[i][:], ps_p[i][:])
                nc.tensor.matmul(
                    ps_v[0:1, :], lhsT2_sb[:, 512:513], rhs2_sb[:],
                    start=False, stop=True,
                )
                v1_sb = const_pool.tile([1, D], BF16, tag="v1")
                nc.vector.tensor_copy(v1_sb[:], ps_v[0:1, :])

            # ---- phase 3: out = x @ P + v ----
            with tc.tile_pool(name="psO", bufs=1, space="PSUM") as psO_pool:
                ps_vb = psO_pool.tile([128, D], F32, tag="vb", bufs=1)
                nc.tensor.matmul(ps_vb[:], ones_row[0:1, :], v1_sb[0:1, :],
                                 start=True, stop=True)
                v_sb = const_pool.tile([128, D], F32, tag="vsb")
                nc.vector.tensor_copy(v_sb[:], ps_vb[:])

                for t in range(NT):
                    xat_t = xat_pool.tile([128, 4, 128], BF16, tag="xat")
                    xdma = nc.scalar.dma_start(xat_t[:], xat_d.ap()[t])
                    # xat prefetch release-shaped behind G so it doesn't
                    # steal bandwidth from the xa stream during warmup
                    add_dep_helper(xdma.ins,
                                   gate_mms[min(NT - 1, 16 + t // 2)].ins,
                                   reason="xat prefetch BW-shaped behind G")
                    ps = psO_pool.tile([128, D], F32, tag="out", bufs=6)
                    for c in range(4):
                        nc.tensor.matmul(
                            ps[:], xat_t[:, c, :], p_sb[c][:],
                            start=(c == 0), stop=(c == 3),
                        )
                    ot = out_pool.tile([128, D], BF16, tag="ot")
                    nc.vector.tensor_add(ot[:], ps[:], v_sb[:])
                    eng = nc.gpsimd if t % 2 == 0 else nc.sync
                    eng.dma_start(out_d.ap()[t * 128:(t + 1) * 128, :], ot[:])

    nc.compile()
    _built[key] = nc
    return nc


def _prep_host(x, Wq1_w, Wq1_b, Wq2_w, Wq2_b, WR_w, WR_b):
    f = np.float32
    W1a = np.concatenate([Wq1_w, Wq1_b[:, None]], axis=1)   # [512, 513]
    W2a = np.concatenate([Wq2_w, Wq2_b[:, None]], axis=1)
    WRa = np.concatenate([WR_w, WR_b[:, None]], axis=1)

    Khat = (W1a.T.astype(np.float64) @ W2a.astype(np.float64))  # [513, 513]
    Rhat = WRa.T.astype(np.float64)                             # [513, 512]

    khatT = np.zeros((4, 128, GP), f)   # Khat^T core row-chunks, col-padded
    khatT_full = Khat.T                 # [513, 513]
    khatT[:, :, :513] = khatT_full[:512].reshape(4, 128, 513).astype(f)
    rhat = np.ascontiguousarray(Rhat[:512].reshape(4, 128, D).astype(f))

    # augmented rank-1 folds (everything touching G_aug's row/col 512):
    #   P_full += u1 (x) rhat_row + khat_col (x) m1row
    sx = x.sum(axis=1, dtype=np.float64)                 # [B, 512]
    sxa = np.concatenate([sx, np.full((B, 1), float(N))], axis=1)  # [B, 513]
    m1row = sxa @ Rhat                                   # [B, 512]
    u1 = np.einsum('ij,bj->bi', Khat[:, :512], sx)       # [B, 513]
    khat_col = Khat[:, 512]                              # [513]
    rhat_row = Rhat[512]                                 # [512]

    lhsT2 = np.zeros((B, 2, GP), f)
    lhsT2[:, 0, :513] = u1.astype(f)
    lhsT2[:, 1, :513] = khat_col.astype(f)[None, :]
    rhs2 = np.zeros((B, 2, D), f)
    rhs2[:, 0] = rhat_row.astype(f)[None, :]
    rhs2[:, 1] = m1row.astype(f)

    # xat[b, t, p, c, j] = x[b, t*128+j, c*128+p] — per-tile [128, 4, 128]
    # lhsT blocks of x^T
    xat = np.ascontiguousarray(
        x.transpose(0, 2, 1)                     # [B, 512, 4096]
         .reshape(B, 4, 128, NT, 128)            # [B, c, p, t, j]
         .transpose(0, 3, 2, 1, 4)               # [B, t, p, c, j]
    ).astype(BF)
    xa = np.ascontiguousarray(x).astype(BF)

    return [
        {"xa": xa[b], "xat": xat[b], "khatT": khatT.astype(BF),
         "rhat": rhat.astype(BF), "lhsT2": lhsT2[b].astype(BF),
         "rhs2": rhs2[b].astype(BF)}
        for b in range(B)
    ]


def kernel(x, Wq1_w, Wq1_b, Wq2_w, Wq2_b, WR_w, WR_b):
    x = np.asarray(x, dtype=np.float32)
    args = [np.asarray(a, dtype=np.float32)
            for a in (Wq1_w, Wq1_b, Wq2_w, Wq2_b, WR_w, WR_b)]
    in_maps = _prep_host(x, *args)

    nc = _build()
    # the axon-tunneled device occasionally starts in a wedged state
    # (NRT_EXEC_UNIT_UNRECOVERABLE) and recovers on the next attempt
    last_err = None
    for attempt in range(3):
        try:
            res = run_bass_kernel_spmd(nc, in_maps, core_ids=list(range(N_CORES)))
            break
        except Exception as e:  # noqa: BLE001
            last_err = e
            import time as _time
            _time.sleep(2.0)
            try:
                import jax
                jax.clear_caches()
            except Exception:
                pass
    else:
        raise last_err
    return np.stack([res.results[b]["out"].astype(np.float32)
                     for b in range(B)])


# revision 8
# speedup vs baseline: 1.3136x; 1.0918x over previous
"""Trainium2 Bass kernel for GCFAgg-style block:
    q1 = x@W1.T+b1; q2 = x@W2.T+b2; r = x@WR.T+br
    out = (q1 @ q2.T) @ r        (per batch, no softmax)

Algebraic restructuring: with x_aug = [x | 1] and W*_aug = [W* | b*],
    out = x_aug @ P_full,  P_full = Khat @ G_aug @ Rhat
where Khat = W1_aug.T @ W2_aug, Rhat = WR_aug.T. The device computes only
the core G = x.T @ x; every term of P_full involving the augmented
row/col of G_aug is a host-computable rank-1 correction folded into one
K=2 matmul per P chunk (lhsT2/rhs2). Final: out = x @ P[0:512] + v.

Phase 1 runs in fp8e4m3 with DoubleRow perf mode (two 128-token planes
contracted per PE instruction — 2x the bf16 rate); G accumulates in f32
PSUM and only the upper block-triangle is computed (lower via PE
transposes). Phases 2/3 run in bf16 (fp8 there would breach the 2e-2
error gate). Output is stored bf16 and upcast on host. End-to-end
~9e-3 max-rel error vs the fp32 reference (gate 2e-2) — validated
against an exact-input numpy simulation of the quantization chain.

Sharding: batch dim B=8, one batch per NeuronCore (data parallel).
Self-contained: hardcodes shapes (x: [8, 4096, 512] f32).
"""
import os
import sys

sys.path.insert(0, "/opt/trn_rl_repo")

import numpy as np
import ml_dtypes

import concourse.bass as bass
import concourse.mybir as mybir
import concourse.tile as tile
from concourse import bacc
from concourse.bass_utils import run_bass_kernel_spmd
from concourse.masks import make_identity
from concourse.tile_rust import add_dep_helper

B = 8          # batch -> one per core
N = 4096       # tokens per batch
D = 512        # model dim
GP = 640       # augmented dim 513 padded to 5*128 (khat col pad)
NT = N // 128  # 32 row tiles
NT2 = NT // 2  # 16 fp8 double-tiles for phase 1
N_CORES = 8

F32 = mybir.dt.float32
BF16 = mybir.dt.bfloat16
F8E4 = mybir.dt.float8e4
DR = mybir.MatmulPerfMode.DoubleRow
BF = ml_dtypes.bfloat16
F8 = ml_dtypes.float8_e4m3

_built = {}


def _build(key="v3"):
    if key in _built:
        return _built[key]

    nc = bacc.Bacc("TRN2", target_bir_lowering=False, debug=False,
                   num_devices=N_CORES)

    xa8_d = nc.dram_tensor("xa8", (NT2, 128, 2, D), F8E4, kind="ExternalInput")
    xat_d = nc.dram_tensor("xat", (NT, 128, 4, 128), BF16,
                           kind="ExternalInput")
    khatT_d = nc.dram_tensor("khatT", (4, 128, GP), BF16, kind="ExternalInput")
    rhat_d = nc.dram_tensor("rhat", (4, 128, D), BF16, kind="ExternalInput")
    lhsT2_d = nc.dram_tensor("lhsT2", (2, GP), BF16, kind="ExternalInput")
    rhs2_d = nc.dram_tensor("rhs2", (2, D), BF16, kind="ExternalInput")
    out_d = nc.dram_tensor("out", (N, D), BF16, kind="ExternalOutput")

    with tile.TileContext(nc) as tc:
        with (
            tc.tile_pool(name="xa", bufs=10) as xa_pool,
            tc.tile_pool(name="xat", bufs=32) as xat_pool,
            tc.tile_pool(name="const", bufs=1) as const_pool,
            tc.tile_pool(name="gsb", bufs=1) as g_pool,
            tc.tile_pool(name="chain", bufs=1) as chain_pool,
            tc.tile_pool(name="outsb", bufs=6) as out_pool,
        ):
            khat_sb = [const_pool.tile([128, GP], BF16, tag=f"khat{c}",
                                       name=f"khat{c}") for c in range(4)]
            rhat_sb = [const_pool.tile([128, D], BF16, tag=f"rhat{c}",
                                       name=f"rhat{c}") for c in range(4)]
            lhsT2_sb = const_pool.tile([2, GP], BF16, tag="lhsT2")
            rhs2_sb = const_pool.tile([2, D], BF16, tag="rhs2")
            ident = const_pool.tile([128, 128], BF16, tag="ident")
            make_identity(nc, ident[:])
            ones_row = const_pool.tile([1, 128], BF16, tag="ones_row")
            nc.vector.memset(ones_row[:], 1.0)
            # tiny K=2 chain constants on the otherwise-idle scalar queue,
            # needed first by the chain's leading K=2 fold
            nc.scalar.dma_start(lhsT2_sb[:], lhsT2_d.ap()[:])
            nc.scalar.dma_start(rhs2_sb[:], rhs2_d.ap()[:])

            # ---- phase 1: G = x^T @ x, fp8 DoubleRow over 16 double-tiles
            # (upper block-triangle only; xa8 streams on two DMA queues) ----
            g_sb = [g_pool.tile([128, D], BF16, tag=f"g{c}", name=f"g{c}")
                    for c in range(4)]
            with tc.tile_pool(name="psG", bufs=1, space="PSUM") as psG_pool:
                ps_ga = [psG_pool.tile([128, D - c * 128], F32, tag=f"ga{c}",
                                       name=f"ga{c}") for c in range(4)]
                gate_mms = []
                for t in range(NT2):
                    xa_t = xa_pool.tile([128, 2, D], F8E4, tag="xa")
                    if t < 2:
                        # split across both queues so the first matmul
                        # starts as early as possible
                        nc.sync.dma_start(xa_t[:, 0, :], xa8_d.ap()[t, :, 0, :])
                        nc.gpsimd.dma_start(xa_t[:, 1, :], xa8_d.ap()[t, :, 1, :])
                    else:
                        eng = nc.sync if t % 2 == 0 else nc.gpsimd
                        eng.dma_start(xa_t[:], xa8_d.ap()[t])
                    for c in range(4):
                        mm = nc.tensor.matmul(
                            ps_ga[c][:],
                            xa_t[:, :, c * 128:(c + 1) * 128],
                            xa_t[:, :, c * 128:D],
                            start=(t == 0), stop=(t == NT2 - 1),
                            perf_mode=DR,
                        )
                        if c == 3:
                            gate_mms.append(mm)
                # bulk chain constants gated behind early G so their
                # triggers queue after the gpsimd xa triggers
                const_dmas = []
                for c in range(4):
                    const_dmas.append(
                        nc.gpsimd.dma_start(rhat_sb[c][:], rhat_d.ap()[c]))
                for c in range(4):
                    const_dmas.append(
                        nc.gpsimd.dma_start(khat_sb[c][:], khatT_d.ap()[c]))
                for cd in const_dmas:
                    add_dep_helper(cd.ins, gate_mms[4].ins,
                                   reason="const loads gated behind G t=4")
                # upper blocks into SBUF (cast f32 PSUM -> bf16), spread
                # across vector/scalar so casts run in parallel (gpsimd
                # cannot read PSUM on TRN2)
                def ps_copy(i, dst, src):
                    if i % 2 == 0:
                        nc.vector.tensor_copy(dst, src)
                    else:
                        nc.scalar.copy(dst, src)

                for c in range(4):
                    ps_copy(c, g_sb[c][:, c * 128:D], ps_ga[c][:])
                # lower blocks = transpose of upper (G symmetric)
                tr_i = 0
                for c2 in range(1, 4):
                    for c1 in range(c2):
                        ps_tr = psG_pool.tile([128, 128], BF16, tag="tr",
                                              bufs=3)
                        nc.tensor.transpose(
                            ps_tr[:],
                            g_sb[c1][:, c2 * 128:(c2 + 1) * 128],
                            ident[:],
                        )
                        ps_copy(tr_i,
                                g_sb[c2][:, c1 * 128:(c1 + 1) * 128], ps_tr[:])
                        tr_i += 1

            # ---- phase 2: P_full = Khat @ G_aug @ Rhat ----
            # K=2 aug fold opens each PSUM group; M1[j] groups are
            # software-pipelined against P's j-steps
            with tc.tile_pool(name="psC", bufs=1, space="PSUM") as psC_pool:
                m1_sb = [chain_pool.tile([128, D], BF16, tag=f"m1{c}",
                                         name=f"m1{c}") for c in range(4)]
                p_sb = [chain_pool.tile([128, D], BF16, tag=f"p{c}",
                                        name=f"p{c}") for c in range(4)]
                ps_p = [psC_pool.tile([128, D], F32, tag=f"pp{c}",
                                      name=f"pp{c}") for c in range(4)]
                ps_v = psC_pool.tile([128, D], F32, tag="pv", name="pv")
                ps_m1 = [psC_pool.tile([128, D], F32, tag="m1ps", bufs=2,
                                       name=f"m1ps{j}") for j in range(4)]

                for i in range(4):
                    nc.tensor.matmul(
                        ps_p[i][:], lhsT2_sb[:, i * 128:(i + 1) * 128],
                        rhs2_sb[:], start=True, stop=False,
                    )
                nc.tensor.matmul(
                    ps_v[0:1, :], lhsT2_sb[:, 512:513], rhs2_sb[:],
                    start=True, stop=False,
                )

                def emit_m1(j):
                    for k in range(4):
                        nc.tensor.matmul(
                            ps_m1[j][:], g_sb[k][:, j * 128:(j + 1) * 128],
                            rhat_sb[k][:], start=(k == 0), stop=(k == 3),
                        )
                    ps_copy(j, m1_sb[j][:], ps_m1[j][:])

                def emit_pstep(j):
                    for i in range(4):
                        nc.tensor.matmul(
                            ps_p[i][:],
                            khat_sb[j][:, i * 128:(i + 1) * 128],
                            m1_sb[j][:], start=False, stop=(j == 3),
                        )
                    nc.tensor.matmul(
                        ps_v[0:1, :], khat_sb[j][:, 512:513], m1_sb[j][:],
                        start=False, stop=(j == 3),
                    )

                emit_m1(0)
                emit_m1(1)
                emit_pstep(0)
                emit_m1(2)
                emit_pstep(1)
                emit_m1(3)
                emit_pstep(2)
                emit_pstep(3)

                for i in range(4):
                    ps_copy(i, p_sb[i][:], ps_p[i][:])
                v1_sb = const_pool.tile([1, D], BF16, tag="v1")
                nc.vector.tensor_copy(v1_sb[:], ps_v[0:1, :])

            # ---- phase 3: out = x @ P + v ----
            with tc.tile_pool(name="psO", bufs=1, space="PSUM") as psO_pool:
                ps_vb = psO_pool.tile([128, D], F32, tag="vb", bufs=1)
                nc.tensor.matmul(ps_vb[:], ones_row[0:1, :], v1_sb[0:1, :],
                                 start=True, stop=True)
                v_sb = const_pool.tile([128, D], F32, tag="vsb")
                nc.vector.tensor_copy(v_sb[:], ps_vb[:])

                for t in range(NT):
                    xat_t = xat_pool.tile([128, 4, 128], BF16, tag="xat")
                    # two prefetch queues: evens on scalar, odds on sync
                    # (sync is free once the short fp8 xa stream is done);
                    # release shaped behind G progress so warmup bandwidth
                    # stays with the xa stream
                    eng = nc.scalar if t % 2 == 0 else nc.sync
                    xdma = eng.dma_start(xat_t[:], xat_d.ap()[t])
                    add_dep_helper(xdma.ins,
                                   gate_mms[min(NT2 - 1, 5 + t // 3)].ins,
                                   reason="xat prefetch shaped behind G")
                    ps = psO_pool.tile([128, D], F32, tag="out", bufs=6)
                    for c in range(4):
                        nc.tensor.matmul(
                            ps[:], xat_t[:, c, :], p_sb[c][:],
                            start=(c == 0), stop=(c == 3),
                        )
                    ot = out_pool.tile([128, D], BF16, tag="ot")
                    nc.vector.tensor_add(ot[:], ps[:], v_sb[:])
                    if t >= NT - 2:
                        # split the final stores across both queues to
                        # shorten the drain tail
                        half = D // 2
                        nc.gpsimd.dma_start(
                            out_d.ap()[t * 128:(t + 1) * 128, 0:half],
                            ot[:, 0:half])
                        nc.sync.dma_start(
                            out_d.ap()[t * 128:(t + 1) * 128, half:D],
                            ot[:, half:D])
                    else:
                        eng = nc.gpsimd if t % 2 == 0 else nc.sync
                        eng.dma_start(out_d.ap()[t * 128:(t + 1) * 128, :],
                                      ot[:])

    nc.compile()
    _built[key] = nc
    return nc


def _prep_host(x, Wq1_w, Wq1_b, Wq2_w, Wq2_b, WR_w, WR_b):
    f = np.float32
    W1a = np.concatenate([Wq1_w, Wq1_b[:, None]], axis=1)   # [512, 513]
    W2a = np.concatenate([Wq2_w, Wq2_b[:, None]], axis=1)
    WRa = np.concatenate([WR_w, WR_b[:, None]], axis=1)

    Khat = (W1a.T.astype(np.float64) @ W2a.astype(np.float64))  # [513, 513]
    Rhat = WRa.T.astype(np.float64)                             # [513, 512]

    khatT = np.zeros((4, 128, GP), f)   # Khat^T core row-chunks, col-padded
    khatT[:, :, :513] = Khat.T[:512].reshape(4, 128, 513).astype(f)
    rhat = np.ascontiguousarray(Rhat[:512].reshape(4, 128, D).astype(f))

    # augmented rank-1 folds (everything touching G_aug's row/col 512):
    #   P_full += u1 (x) rhat_row + khat_col (x) m1row
    sx = x.sum(axis=1, dtype=np.float64)                 # [B, 512]
    sxa = np.concatenate([sx, np.full((B, 1), float(N))], axis=1)  # [B, 513]
    m1row = sxa @ Rhat                                   # [B, 512]
    u1 = np.einsum('ij,bj->bi', Khat[:, :512], sx)       # [B, 513]

    lhsT2 = np.zeros((B, 2, GP), f)
    lhsT2[:, 0, :513] = u1.astype(f)
    lhsT2[:, 1, :513] = Khat[:, 512].astype(f)[None, :]
    rhs2 = np.zeros((B, 2, D), f)
    rhs2[:, 0] = Rhat[512].astype(f)[None, :]
    rhs2[:, 1] = m1row.astype(f)

    # fp8 phase-1 layout: xa8[b, t2, p, h, d] = x[b, t2*256 + h*128 + p, d]
    xa8 = np.ascontiguousarray(
        x.astype(F8).reshape(B, NT2, 2, 128, D).transpose(0, 1, 3, 2, 4))

    # xat[b, t, p, c, j] = x[b, t*128+j, c*128+p] — per-tile [128, 4, 128]
    # lhsT blocks of x^T
    xat = np.ascontiguousarray(
        x.transpose(0, 2, 1)                     # [B, 512, 4096]
         .reshape(B, 4, 128, NT, 128)            # [B, c, p, t, j]
         .transpose(0, 3, 2, 1, 4)               # [B, t, p, c, j]
    ).astype(BF)

    return [
        {"xa8": xa8[b], "xat": xat[b], "khatT": khatT.astype(BF),
         "rhat": rhat.astype(BF), "lhsT2": lhsT2[b].astype(BF),
         "rhs2": rhs2[b].astype(BF)}
        for b in range(B)
    ]


def kernel(x, Wq1_w, Wq1_b, Wq2_w, Wq2_b, WR_w, WR_b):
    x = np.asarray(x, dtype=np.float32)
    args = [np.asarray(a, dtype=np.float32)
            for a in (Wq1_w, Wq1_b, Wq2_w, Wq2_b, WR_w, WR_b)]
    in_maps = _prep_host(x, *args)

    nc = _build()
    # the axon-tunneled device occasionally starts in a wedged state
    # (NRT_EXEC_UNIT_UNRECOVERABLE) and recovers on the next attempt
    last_err = None
    for attempt in range(3):
        try:
            res = run_bass_kernel_spmd(nc, in_maps, core_ids=list(range(N_CORES)))
            break
        except Exception as e:  # noqa: BLE001
            last_err = e
            import time as _time
            _time.sleep(2.0)
            try:
                import jax
                jax.clear_caches()
            except Exception:
                pass
    else:
        raise last_err
    return np.stack([res.results[b]["out"].astype(np.float32)
                     for b in range(B)])
